# revision 1
# baseline (speedup 1.0000x reference)
"""Trainium2 Bass kernel for a 3-layer GAT (gnn_message_passing).

Strategy (8 NeuronCores):
- Nodes are relabeled and dealt (sorted by in-degree, round-robin) into
  128-node "windows"; windows are dealt to the 8 cores. Each core owns
  its windows' nodes and ALL edges incident to them (dst-sharded).
- Per layer: each core projects its node slice (x @ Wext, where Wext also
  produces the per-node attention scalars a_src/a_dst), writes a node
  table slice, and the slices are AllGathered so every core holds the
  full [h | a_src | a_dst] table.
- Edge phase: per window, gather h|a_src rows of edge sources via
  dma_gather (int16 indices; table split in two <=32768-row halves),
  gather a_dst of edge destinations from a core-local table, compute
  w_e = exp(leakyrelu(a_src+a_dst)) per edge, scale messages, and
  scatter-add into the window's 128 nodes with a one-hot matmul
  (Q[e, n] = [dst_loc[e] == n]) accumulating in PSUM. The softmax
  denominator is accumulated as an extra matmul column (exp-sum per
  node), so no segment-max pass is needed (exponents are O(10), safe).
- Global mean-pool is a one-hot matmul over graph ids + AllReduce, then
  the final linear layer on-device. Core 0's output is returned.
"""

import numpy as np

import concourse.bacc as bacc
import concourse.bass as bass
import concourse.mybir as mybir
from concourse.masks import make_identity
from concourse.tile import TileContext
from concourse.bass_utils import run_bass_kernel_spmd

F32 = mybir.dt.float32
I16 = mybir.dt.int16
I32 = mybir.dt.int32

NCORES = 8
P = 128
NEG_SLOPE = 0.2
NUM_CLASSES = 10
HEADS = 4
C = 64


# ----------------------------------------------------------------------------
# Host-side preprocessing (sharding)
# ----------------------------------------------------------------------------

def _wrap16(v):
    """[n] int -> [128, n/16] int16 layout for dma_gather indices."""
    a = v.reshape(-1, 16).T
    return np.tile(a, (8, 1)).astype(np.int16)


def _preprocess(x_ids, degrees, edge_src, edge_dst, batch, num_graphs):
    N = x_ids.shape[0]
    src = np.concatenate([edge_src, np.arange(N)]).astype(np.int64)
    dst = np.concatenate([edge_dst, np.arange(N)]).astype(np.int64)

    total_w = -(-N // P)                      # windows overall
    WPC = -(-total_w // NCORES)               # windows per core
    SLOTS = WPC * P                           # node slots per core
    NROWS = NCORES * SLOTS                    # table rows

    indeg = np.bincount(dst, minlength=N)
    order = np.argsort(-indeg, kind="stable")
    nwin = WPC * NCORES
    # deal sorted nodes round-robin into nwin windows -> balanced loads
    win_of = np.empty(N, np.int64)
    slot_of = np.empty(N, np.int64)
    win_of[order] = np.arange(N) % nwin
    slot_of[order] = np.arange(N) // nwin
    # deal windows (sorted by load) round-robin onto cores
    wload = np.zeros(nwin, np.int64)
    np.add.at(wload, win_of[dst], 1)
    worder = np.argsort(-wload, kind="stable")
    core_of_w = np.empty(nwin, np.int64)
    wloc_of_w = np.empty(nwin, np.int64)
    core_of_w[worder] = np.arange(nwin) % NCORES
    wloc_of_w[worder] = np.arange(nwin) // NCORES

    core_of = core_of_w[win_of]
    wloc_of = wloc_of_w[win_of]
    newrow = core_of * SLOTS + wloc_of * P + slot_of  # global table row per node

    # avoid gather overrun past table end: guaranteed by padded row design
    esrc_row = newrow[src]
    ecore = core_of[dst]
    ewloc = wloc_of[dst]
    eslot = slot_of[dst]

    HALF = 32768
    nhalves = 2 if NROWS > HALF else 1
    ehalf = (esrc_row >= HALF).astype(np.int64) if nhalves == 2 else np.zeros(len(src), np.int64)

    # group sizes per (core, window, half)
    gkey = (ecore * WPC + ewloc) * 2 + ehalf
    gcnt = np.bincount(gkey, minlength=NCORES * WPC * 2).reshape(NCORES, WPC, 2)
    C0 = int(-(-gcnt[:, :, 0].max() // P) * P)
    C1 = int(-(-gcnt[:, :, 1].max() // P) * P) if nhalves == 2 else 0
    C0 = max(C0, P)
    if nhalves == 2:
        C1 = max(C1, P)
    NB0, NB1 = C0 // P, C1 // P
    NB = NB0 + NB1
    CW = C0 + C1                               # padded edges per window

    eorder = np.lexsort((ehalf, ewloc, ecore))  # stable grouping

    per_core = []
    for k in range(NCORES):
        tab_idx = np.zeros(WPC * CW, np.int64)      # gather idx into table half
        dst_loc = np.full(WPC * CW, -1, np.int64)   # slot within window, -1 pad
        sel_core = eorder[ecore[eorder] == k]
        for w in range(WPC):
            sel_w = sel_core[ewloc[sel_core] == w]
            base = w * CW
            for h in range(nhalves):
                e = sel_w[ehalf[sel_w] == h]
                cap = C0 if h == 0 else C1
                off = base if h == 0 else base + C0
                assert len(e) <= cap
                rows = esrc_row[e] - (HALF if h == 1 else 0)
                tab_idx[off:off + len(e)] = rows
                dst_loc[off:off + len(e)] = eslot[e]

        # per-core node data in new order
        nodes_mask = core_of == np.int64(k)
        nodes = np.nonzero(nodes_mask)[0]
        loc = wloc_of[nodes] * P + slot_of[nodes]
        xi = np.zeros(SLOTS, np.int64)
        dg = np.zeros((SLOTS, 2), np.float32)
        gi = np.full(SLOTS, -1, np.int64)
        xi[loc] = np.asarray(x_ids)[nodes]
        dg[loc] = np.asarray(degrees, np.float32)[nodes]
        gi[loc] = np.asarray(batch)[nodes]

        import ml_dtypes
        per_core.append(dict(
            tab_idx=_wrap16(tab_idx),
            dstrow=dst_loc.reshape(WPC, CW).astype(ml_dtypes.bfloat16),  # [WPC, CW]
            dst_loc=dst_loc.reshape(-1, P).T.astype(np.int32).copy(),  # [128, WPC*NB]
            gid=gi.reshape(WPC, P).T.astype(np.int32).copy(),          # [128, WPC]
            xids=_wrap16(xi),
            deg=dg,
        ))

    cfg = dict(N=N, WPC=WPC, SLOTS=SLOTS, NROWS=NROWS, nhalves=nhalves,
               C0=C0, C1=C1, NB0=NB0, NB1=NB1, NB=NB, CW=CW,
               num_graphs=num_graphs)
    return per_core, cfg


def _prep_weights(emb, W1, as1, ad1, W2, as2, ad2, W3, as3, ad3, b1, b2, b3,
                  linW, linb):
    """Fold attention vectors into projection matrices (host-side)."""
    def ext(W, a_s, a_d):
        # W: [H*C, d_in]; a_s/a_d: [H, C] -> Wext [d_in, H*C + 2H]
        Wt = np.asarray(W, np.float32).T                 # [d_in, H*C]
        H = a_s.shape[0]
        d_in = Wt.shape[0]
        was = np.zeros((d_in, H), np.float32)
        wad = np.zeros((d_in, H), np.float32)
        for h in range(H):
            was[:, h] = Wt[:, h * C:(h + 1) * C] @ np.asarray(a_s, np.float32)[h]
            wad[:, h] = Wt[:, h * C:(h + 1) * C] @ np.asarray(a_d, np.float32)[h]
        return np.concatenate([Wt, was, wad], axis=1)

    VOCAB, EMB = emb.shape
    emb_ext = np.zeros((VOCAB, 64), np.float32)
    emb_ext[:, :EMB] = np.asarray(emb, np.float32)

    return dict(
        emb_ext=emb_ext,
        w1=ext(W1, as1, ad1),                 # [64, 264]
        w2=ext(W2, as2, ad2),                 # [256, 264]
        w3=ext(W3, as3, ad3),                 # [256, 66]
        b1=np.tile(np.asarray(b1, np.float32)[None, :], (P, 1)),
        b2=np.tile(np.asarray(b2, np.float32)[None, :], (P, 1)),
        b3=np.tile(np.asarray(b3, np.float32)[None, :], (P, 1)),
        linwt=np.asarray(linW, np.float32).T.copy(),      # [C, 10]
        linb=np.tile(np.asarray(linb, np.float32)[None, :], (64, 1)),
    )


# ----------------------------------------------------------------------------
# Kernel builder
# ----------------------------------------------------------------------------

def _build(cfg, reps=1, variant=()):
    var = set(variant)
    WPC, SLOTS, NROWS = cfg["WPC"], cfg["SLOTS"], cfg["NROWS"]
    nhalves, C0, C1 = cfg["nhalves"], cfg["C0"], cfg["C1"]
    NB0, NB1, NB, CW = cfg["NB0"], cfg["NB1"], cfg["NB"], cfg["CW"]
    NG = cfg["num_graphs"]
    VOCAB = cfg["VOCAB"]
    BF16 = mybir.dt.bfloat16
    TW = 384                                  # table row width in bf16 (768B)
    HALF = 32768

    nc = bacc.Bacc("TRN2", target_bir_lowering=False, debug=False,
                   num_devices=NCORES, num_swdge_queues=4)

    # ---- DRAM tensors ----
    din = {}
    din["emb_ext"] = nc.dram_tensor("emb_ext", [VOCAB, 64], F32, kind="ExternalInput")
    din["xids"] = nc.dram_tensor("xids", [P, SLOTS // 16], I16, kind="ExternalInput")
    din["deg"] = nc.dram_tensor("deg", [SLOTS, 2], F32, kind="ExternalInput")
    din["tab_idx"] = nc.dram_tensor("tab_idx", [P, WPC * CW // 16], I16, kind="ExternalInput")
    din["dstrow"] = nc.dram_tensor("dstrow", [WPC, CW], BF16, kind="ExternalInput")
    din["dst_loc"] = nc.dram_tensor("dst_loc", [P, WPC * NB], I32, kind="ExternalInput")
    din["gid"] = nc.dram_tensor("gid", [P, WPC], I32, kind="ExternalInput")
    din["w1"] = nc.dram_tensor("w1", [64, 264], F32, kind="ExternalInput")
    din["w2"] = nc.dram_tensor("w2", [256, 264], F32, kind="ExternalInput")
    din["w3"] = nc.dram_tensor("w3", [256, 66], F32, kind="ExternalInput")
    din["b1"] = nc.dram_tensor("b1", [P, 256], F32, kind="ExternalInput")
    din["b2"] = nc.dram_tensor("b2", [P, 256], F32, kind="ExternalInput")
    din["b3"] = nc.dram_tensor("b3", [P, 64], F32, kind="ExternalInput")
    din["linwt"] = nc.dram_tensor("linwt", [64, NUM_CLASSES], F32, kind="ExternalInput")
    din["linb"] = nc.dram_tensor("linb", [64, NUM_CLASSES], F32, kind="ExternalInput")

    cc_in = nc.dram_tensor("cc_in", [SLOTS, TW], BF16, kind="Internal")
    table = nc.dram_tensor("table", [NROWS, TW], BF16, kind="Internal",
                           addr_space="Shared")
    adtab = nc.dram_tensor("adtab", [SLOTS, 4], BF16, kind="Internal")
    qa_d = nc.dram_tensor("qa_d", [WPC, P, NB * P], BF16, kind="Internal")
    qn_d = nc.dram_tensor("qn_d", [WPC, P, NB * P], BF16, kind="Internal")
    ar_in = nc.dram_tensor("ar_in", [64, 65], F32, kind="Internal")
    ar_out = nc.dram_tensor("ar_out", [64, 65], F32, kind="Internal",
                            addr_space="Shared")
    out = nc.dram_tensor("out", [NG, NUM_CLASSES], F32, kind="ExternalOutput")

    rg = [list(range(NCORES))]

    LAYERS = [
        dict(d_in=64, HC=256, heads=4, wname="w1", bname="b1", tcols=264, gelem=TW),
        dict(d_in=256, HC=256, heads=4, wname="w2", bname="b2", tcols=264, gelem=TW),
        dict(d_in=256, HC=64, heads=1, wname="w3", bname="b3", tcols=66, gelem=128),
    ]

    with TileContext(nc) as tc:
        with tc.tile_pool(name="const", bufs=1) as cpool, \
             tc.tile_pool(name="xres", bufs=1) as xpool, \
             tc.tile_pool(name="proj", bufs=3) as ppool, \
             tc.tile_pool(name="edge", bufs=2) as epool, \
             tc.tile_pool(name="small", bufs=3) as spool, \
             tc.tile_pool(name="psA", bufs=2, space="PSUM") as psA, \
             tc.tile_pool(name="psB", bufs=2, space="PSUM") as psB, \
             tc.tile_pool(name="psC", bufs=2, space="PSUM") as psC, \
             tc.tile_pool(name="psD", bufs=1, space="PSUM") as psD:

            # ---- constants ----
            ident = cpool.tile([P, P], F32, tag="ident")
            make_identity(nc, ident[:])
            iota_r = cpool.tile([P, P], I32, tag="iota")
            nc.gpsimd.iota(iota_r[:], pattern=[[1, P]], base=0, channel_multiplier=0)

            tab_idx = cpool.tile([P, WPC * CW // 16], I16, tag="tabidx")
            nc.sync.dma_start(out=tab_idx[:], in_=din["tab_idx"][:])
            ones_bf = cpool.tile([1, P], BF16, tag="onesbf")
            nc.vector.memset(ones_bf[:], 1.0)
            iota_c = cpool.tile([P, 1], I32, tag="iotac")
            nc.gpsimd.iota(iota_c[:], pattern=[[0, 1]], base=0, channel_multiplier=1)
            dst_loc = cpool.tile([P, WPC * NB], I32, tag="dstloc")
            nc.sync.dma_start(out=dst_loc[:], in_=din["dst_loc"][:])
            gid_t = cpool.tile([P, WPC], I32, tag="gid")
            nc.sync.dma_start(out=gid_t[:], in_=din["gid"][:])
            xids_t = cpool.tile([P, SLOTS // 16], I16, tag="xids")
            nc.sync.dma_start(out=xids_t[:], in_=din["xids"][:])

            wts = {}
            for nm, rows, cols in (("w1", 64, 264), ("w2", 256, 264), ("w3", 256, 66)):
                nk = -(-rows // P)
                tl = []
                for kc in range(nk):
                    t = cpool.tile([P, cols], F32, tag=f"{nm}_{kc}")
                    r0, r1 = kc * P, min((kc + 1) * P, rows)
                    nc.sync.dma_start(out=t[: r1 - r0, :], in_=din[nm][r0:r1, :])
                    tl.append(t)
                wts[nm] = tl
            bias = {}
            for nm, cols in (("b1", 256), ("b2", 256), ("b3", 64)):
                t = cpool.tile([P, cols], F32, tag=nm)
                nc.sync.dma_start(out=t[:], in_=din[nm][:])
                bias[nm] = t
            linwt = cpool.tile([64, NUM_CLASSES], F32, tag="linwt")
            nc.sync.dma_start(out=linwt[:], in_=din["linwt"][:])
            linb = cpool.tile([64, NUM_CLASSES], F32, tag="linb")
            nc.sync.dma_start(out=linb[:], in_=din["linb"][:])

            # ---- resident activations ----
            xbuf = xpool.tile([P, WPC * 256], F32, tag="xbuf")
            x3 = xpool.tile([P, WPC * 64], F32, tag="x3")

            for _rep in range(reps):
             for il, L in enumerate(LAYERS):
                 d_in, HC, heads = L["d_in"], L["HC"], L["heads"]
                 tcols, gelem = L["tcols"], L["gelem"]
                 wt = wts[L["wname"]]
                 bt = bias[L["bname"]]

                 # ================= projection =================
                 for t in range(WPC):
                     projp = psA.tile([P, tcols], F32, tag="proj")
                     if il == 0:
                         embG = ppool.tile([P, 1, 64], F32, tag="embG")
                         nc.gpsimd.dma_gather(
                             embG[:], din["emb_ext"][:],
                             xids_t[:, t * 8:(t + 1) * 8],
                             num_idxs=P, num_idxs_reg=P, elem_size=64,
                             single_packet=False)
                         nc.sync.dma_start(out=embG[:, 0, 62:64],
                                           in_=din["deg"][t * P:(t + 1) * P, :])
                         chunks = [embG[:, 0, 0:64]]
                     else:
                         xw = xbuf[:, t * 256:(t + 1) * 256]
                         chunks = [xw[:, 0:128], xw[:, 128:256]]
                     for kc, xc in enumerate(chunks):
                         dk = xc.shape[1]
                         xtp = psB.tile([P, P], F32, tag="xT")
                         nc.tensor.transpose(xtp[:dk, :P], xc, ident[:])
                         xts = ppool.tile([P, P], F32, tag="xTs")
                         nc.vector.tensor_copy(xts[:dk, :], xtp[:dk, :])
                         nc.tensor.matmul(projp[:], lhsT=xts[:dk, :P],
                                          rhs=wt[kc][:dk, :tcols],
                                          start=(kc == 0),
                                          stop=(kc == len(chunks) - 1))
                     trow = ppool.tile([P, TW], BF16, tag="trow")
                     nc.vector.tensor_copy(trow[:, 0:HC], projp[:, 0:HC])
                     nc.vector.tensor_copy(
                         trow[:, HC:HC + 2 * heads].bitcast(F32),
                         projp[:, HC:HC + heads])
                     nc.sync.dma_start(
                         out=cc_in[t * P:(t + 1) * P, 0:HC + 2 * heads],
                         in_=trow[:, 0:HC + 2 * heads])
                     # a_dst columns -> local adtab (bf16)
                     adrow = ppool.tile([P, 4], BF16, tag="adrow")
                     nc.vector.tensor_copy(adrow[:, 0:heads],
                                           projp[:, HC + heads:HC + 2 * heads])
                     nc.sync.dma_start(
                         out=adtab[t * P:(t + 1) * P, 0:heads],
                         in_=adrow[:, 0:heads])

                 # ================= allgather =================
                 if "nocc" not in var:
                     nc.gpsimd.collective_compute(
                         "AllGather", mybir.AluOpType.bypass, replica_groups=rg,
                         ins=[cc_in[:, :]], outs=[table[:, :]])

                 # ================= edge phase =================
                 mc = HC + heads                # message cols (msg | w)
                 ADE_OFF = 280                  # adE columns in opsum bank
                 for t in range(WPC):
                     Gt = epool.tile([P, NB, gelem], BF16, tag="G")
                     ib = t * CW // 16
                     if "nogather" not in var:
                         h0tab = table[0:min(HALF, NROWS), 0:gelem]
                         CA = (NB0 // 2) * P
                         if CA:
                             nc.gpsimd.dma_gather(
                                 Gt[:, 0:NB0 // 2, :], h0tab,
                                 tab_idx[:, ib:ib + CA // 16],
                                 num_idxs=CA, num_idxs_reg=CA,
                                 elem_size=gelem, elem_step=TW,
                                 single_packet=False, queue_num=0)
                         nc.gpsimd.dma_gather(
                             Gt[:, NB0 // 2:NB0, :], h0tab,
                             tab_idx[:, ib + CA // 16:ib + C0 // 16],
                             num_idxs=C0 - CA, num_idxs_reg=C0 - CA,
                             elem_size=gelem, elem_step=TW,
                             single_packet=False, queue_num=2)
                     if nhalves == 2 and "nogather" not in var:
                         h1tab = table[HALF:NROWS, 0:gelem]
                         CB = (NB1 // 2) * P
                         if CB:
                             nc.gpsimd.dma_gather(
                                 Gt[:, NB0:NB0 + NB1 // 2, :], h1tab,
                                 tab_idx[:, ib + C0 // 16:ib + (C0 + CB) // 16],
                                 num_idxs=CB, num_idxs_reg=CB,
                                 elem_size=gelem, elem_step=TW,
                                 single_packet=False, queue_num=1)
                         nc.gpsimd.dma_gather(
                             Gt[:, NB0 + NB1 // 2:NB, :], h1tab,
                             tab_idx[:, ib + (C0 + CB) // 16:ib + CW // 16],
                             num_idxs=C1 - CB, num_idxs_reg=C1 - CB,
                             elem_size=gelem, elem_step=TW,
                             single_packet=False, queue_num=3)
                     # window dst-slot row + a_dst vector (contiguous loads)
                     drow = spool.tile([1, CW], BF16, tag="drow")
                     nc.sync.dma_start(out=drow[:], in_=din["dstrow"][t:t + 1, :])
                     adWin = spool.tile([P, 4], BF16, tag="adWin")
                     nc.sync.dma_start(out=adWin[:, 0:heads],
                                       in_=adtab[t * P:(t + 1) * P, 0:heads])

                     if "noedge" in var:
                         xdst0 = (x3[:, t * 64:(t + 1) * 64] if il == 2
                                  else xbuf[:, t * 256:(t + 1) * 256])
                         nc.vector.memset(xdst0, 0.0)
                         continue
                     opsum = psC.tile([P, ADE_OFF + NB * 4], F32, tag="edge")
                     # Qn[n, e] = [dst_loc[e] == n], built without a transpose:
                     # PE outer-product broadcasts dstrow across partitions,
                     # DVE is_equal against the partition index -> SBUF.
                     # Built in layer 0 only; reloaded from DRAM afterwards.
                     QnS = epool.tile([P, NB * P], BF16, tag="Qn")
                     if il > 0:
                         nc.sync.dma_start(out=QnS[:], in_=qn_d[t, :, :])
                     else:
                      nch = -(-CW // 512)
                      for ch in range(nch):
                         c0 = ch * 512
                         cw = min(512, CW - c0)
                         qnp = psB.tile([P, 512], F32, tag="xT")
                         nc.tensor.matmul(qnp[:, 0:cw], lhsT=ones_bf[0:1, :],
                                          rhs=drow[0:1, c0:c0 + cw],
                                          start=True, stop=True)
                         ic_ap = bass.AP(iota_c[:].tensor, iota_c[:].offset,
                                         [list(iota_c[:].ap[0]), [0, cw]])
                         nc.vector.tensor_tensor(out=QnS[:, c0:c0 + cw],
                                                 in0=qnp[:, 0:cw], in1=ic_ap,
                                                 op=mybir.AluOpType.is_equal)
                      nc.sync.dma_start(out=qn_d[t, :, :], in_=QnS[:])
                     # per-edge a_dst = Qn_b.T @ adWin -> opsum[:, ADE_OFF+...]
                     for b in range(NB):
                         nc.tensor.matmul(
                             opsum[:, ADE_OFF + b * 4:ADE_OFF + b * 4 + heads],
                             lhsT=QnS[:, b * P:(b + 1) * P],
                             rhs=adWin[:, 0:heads], start=True, stop=True)
                     # edge weights w = exp(leakyrelu(a_src + a_dst))
                     sm = spool.tile([P, NB * heads], F32, tag="sm")
                     wte = spool.tile([P, NB * heads], F32, tag="wte")
                     ade_ap = bass.AP(opsum[:, 0:1].tensor, opsum[:, 0:1].offset + ADE_OFF,
                                      [list(opsum[:, 0:1].ap[0]), [4, NB], [1, heads]])
                     nc.vector.tensor_tensor(
                         out=sm[:], in0=Gt[:, :, HC:HC + 2 * heads].bitcast(F32),
                         in1=ade_ap, op=mybir.AluOpType.add)
                     we2 = spool.tile([P, NB * heads], F32, tag="we2")
                     nc.scalar.activation(wte[:], sm[:],
                                          mybir.ActivationFunctionType.Exp)
                     nc.scalar.activation(we2[:], sm[:],
                                          mybir.ActivationFunctionType.Exp,
                                          scale=NEG_SLOPE)
                     nc.vector.tensor_tensor(out=wte[:], in0=wte[:], in1=we2[:],
                                             op=mybir.AluOpType.max)
                     # place w into G's a_src columns (becomes matmul rhs col)
                     nc.vector.tensor_copy(Gt[:, :, HC:HC + heads],
                                           wte[:].rearrange("p (b h) -> p b h", b=NB))

                     # one fat op: scale all blocks' message channels by the
                     # per-edge, per-head weight (broadcast C times per head)
                     g00 = Gt[:, 0, 0:1]
                     pstep = g00.ap[0][0]
                     goff = g00.offset
                     msg_ap = bass.AP(g00.tensor, goff,
                                      [[pstep, P], [gelem, NB], [C, heads], [1, C]])
                     wb_ap = bass.AP(g00.tensor, goff + HC,
                                     [[pstep, P], [gelem, NB], [1, heads], [0, C]])
                     nc.vector.tensor_tensor(out=msg_ap, in0=msg_ap, in1=wb_ap,
                                             op=mybir.AluOpType.mult)

                     # one fat op: all blocks' one-hot matrices
                     # (layer 0 builds; later layers reload from DRAM)
                     Qa = epool.tile([P, NB, P], BF16, tag="Qa")
                     if il > 0:
                         nc.sync.dma_start(out=Qa[:], in_=qa_d[t, :, :])
                     else:
                         dl0 = dst_loc[:, t * NB:t * NB + 1]
                         dl_ap = bass.AP(dl0.tensor, dl0.offset,
                                         [list(dl0.ap[0]), [1, NB], [0, P]])
                         io_ap = bass.AP(iota_r[:].tensor, iota_r[:].offset,
                                         [list(iota_r[:].ap[0]), [0, NB], [1, P]])
                         nc.vector.tensor_tensor(out=Qa[:], in0=dl_ap, in1=io_ap,
                                                 op=mybir.AluOpType.is_equal)
                         nc.sync.dma_start(out=qa_d[t, :, :], in_=Qa[:, :, :])

                     for b in range(NB):
                         nc.tensor.matmul(opsum[:, 0:mc], lhsT=Qa[:, b, :],
                                          rhs=Gt[:, b, 0:mc],
                                          start=(b == 0), stop=(b == NB - 1))

                     # finalize: x = relu(msg / denom + bias)
                     dmax = spool.tile([P, heads], F32, tag="dmax")
                     nc.vector.tensor_scalar_max(dmax[:], opsum[:, HC:HC + heads], 1e-30)
                     rec = spool.tile([P, heads], F32, tag="rec")
                     nc.vector.reciprocal(rec[:], dmax[:])
                     ftmp = spool.tile([P, HC], F32, tag="ftmp")
                     r0 = rec[:, 0:1]
                     rb_ap = bass.AP(r0.tensor, r0.offset,
                                     [list(r0.ap[0]), [1, heads], [0, C]])
                     nc.vector.tensor_tensor(out=ftmp[:], in0=opsum[:, 0:HC],
                                             in1=rb_ap, op=mybir.AluOpType.mult)
                     nc.vector.tensor_tensor(out=ftmp[:], in0=ftmp[:], in1=bt[:, 0:HC],
                                             op=mybir.AluOpType.add)
                     xdst = (x3[:, t * 64:(t + 1) * 64] if il == 2
                             else xbuf[:, t * 256:(t + 1) * 256])
                     nc.scalar.activation(xdst, ftmp[:],
                                          mybir.ActivationFunctionType.Relu)

             # ================= pooling + head =================
             gpsum = psD.tile([64, 65], F32, tag="pool")
             for t in range(WPC):
                 prhs = spool.tile([P, 65], F32, tag="prhs")
                 nc.vector.tensor_copy(prhs[:, 0:64], x3[:, t * 64:(t + 1) * 64])
                 nc.vector.memset(prhs[:, 64:65], 1.0)
                 Qg = spool.tile([P, 64], F32, tag="Qg")
                 nc.vector.tensor_tensor(
                     out=Qg[:], in0=gid_t[:, t:t + 1].to_broadcast([P, 64]),
                     in1=iota_r[:, 0:64], op=mybir.AluOpType.is_equal)
                 nc.tensor.matmul(gpsum[:], lhsT=Qg[:], rhs=prhs[:],
                                  start=(t == 0), stop=(t == WPC - 1))
             gsum = spool.tile([64, 65], F32, tag="gsum")
             nc.vector.tensor_copy(gsum[:], gpsum[:])
             nc.sync.dma_start(out=ar_in[:], in_=gsum[:])
             if "nocc" not in var:
                 nc.gpsimd.collective_compute(
                     "AllReduce", mybir.AluOpType.add, replica_groups=rg,
                     ins=[ar_in[:, :]], outs=[ar_out[:, :]])
             pl = spool.tile([64, 65], F32, tag="pl")
             nc.sync.dma_start(out=pl[:], in_=ar_out[:])
             cnt = spool.tile([64, 1], F32, tag="cnt")
             nc.vector.tensor_scalar_max(cnt[:], pl[:, 64:65], 1.0)
             crec = spool.tile([64, 1], F32, tag="crec")
             nc.vector.reciprocal(crec[:], cnt[:])
             pooled = spool.tile([64, 64], F32, tag="pooled")
             nc.vector.tensor_scalar_mul(pooled[:], pl[:, 0:64], crec[:, 0:1])
             ptp = psB.tile([P, P], F32, tag="xT")
             nc.tensor.transpose(ptp[:64, :64], pooled[:], ident[:64, :64])
             pts = spool.tile([64, 64], F32, tag="pts")
             nc.vector.tensor_copy(pts[:], ptp[:64, :64])
             lg = psA.tile([NG, NUM_CLASSES], F32, tag="proj")
             nc.tensor.matmul(lg[:], lhsT=pts[:64, 0:NG],
                              rhs=linwt[:64, :], start=True, stop=True)
             lgs = spool.tile([NG, NUM_CLASSES], F32, tag="lgs")
             nc.vector.tensor_tensor(out=lgs[:], in0=lg[:], in1=linb[0:NG, :],
                                     op=mybir.AluOpType.add)
             nc.sync.dma_start(out=out[:], in_=lgs[:])

    nc.compile()
    return nc


# ----------------------------------------------------------------------------
# Entry point
# ----------------------------------------------------------------------------

LAST_RESULTS = None


def kernel(x_ids, degrees, edge_src, edge_dst, batch, emb,
           W1, as1, ad1, b1, W2, as2, ad2, b2, W3, as3, ad3, b3, linW, linb,
           num_graphs=64, _trace=False):
    x_ids = np.asarray(x_ids)
    per_core, cfg = _preprocess(x_ids, np.asarray(degrees),
                                np.asarray(edge_src), np.asarray(edge_dst),
                                np.asarray(batch), num_graphs)
    wd = _prep_weights(np.asarray(emb), W1, as1, ad1, W2, as2, ad2,
                       W3, as3, ad3, b1, b2, b3, linW, linb)
    cfg["VOCAB"] = wd["emb_ext"].shape[0]

    nc = _build(cfg)

    in_maps = []
    for k in range(NCORES):
        m = dict(per_core[k])
        m["emb_ext"] = wd["emb_ext"]
        m["w1"], m["w2"], m["w3"] = wd["w1"], wd["w2"], wd["w3"]
        m["b1"], m["b2"], m["b3"] = wd["b1"], wd["b2"], wd["b3"]
        m["linwt"], m["linb"] = wd["linwt"], wd["linb"]
        in_maps.append(m)

    global LAST_RESULTS
    res = run_bass_kernel_spmd(nc, in_maps, core_ids=list(range(NCORES)),
                               trace=_trace)
    LAST_RESULTS = res
    return res.results[0]["out"]

